# revision 1
# baseline (speedup 1.0000x reference)
"""Cached self-attention Trainium2 kernel (v3).

Sharding: 8 cores = 2 batches x 4 head-groups. Core c: batch b=c//4, group
g=c%4 owns heads 4g..4g+3 (columns 512g:512g+512 of the q/k/v projections).
Each core projects q/k/v for its heads over the full sequence, runs attention
for its 4 heads, the 4 cores of a batch AllGather the (normalized, transposed)
per-head attention outputs, and each core computes the output projection onto
its 512-column slice of wo (full sequence), so outputs tile the model dim.

All matmuls fp16 x fp16 -> fp32 PSUM. Softmax: exp on ScalarE with the
1/sqrt(128) scale folded into the q evacuation; Z via fp16 DVE running adds +
ones-matmul cross-partition sum + fast reciprocal; normalization applied
during PSUM evacuation of the attention output.
"""
import numpy as np
from contextlib import ExitStack

import concourse.bass as bass
import concourse.tile as tile
from concourse import bacc, mybir
from concourse.bass_utils import run_bass_kernel_spmd

B, S, PC, D, H = 2, 2048, 2048, 2048, 16
HD = D // H            # 128 head dim
GH = H // 4            # 4 heads per core
DG = GH * HD           # 512 head-dims per core
NB = 512               # block size
NKC = (PC + S) // HD   # 32 key chunks of 128
NDC = D // HD          # 16 contraction chunks
F16 = mybir.dt.float16
F32 = mybir.dt.float32
AF = mybir.ActivationFunctionType
ALU = mybir.AluOpType
INV_SQRT_HD = float(1.0 / np.sqrt(HD))

GROUPS = [[0, 1, 2, 3], [4, 5, 6, 7]]


def build():
    nc = bacc.Bacc("TRN2", target_bir_lowering=False, debug=False, num_devices=8)

    def inp(name, shape):
        return nc.dram_tensor(name, shape, F16, kind="ExternalInput").ap()

    xT = inp("xT", [D, S])          # x[b].T
    wq = inp("wq", [D, DG])         # wq[:, 512g:512g+512]
    bq = inp("bq", [DG])            # bq slice / sqrt(HD)
    wk = inp("wk", [D, DG])
    bk = inp("bk", [DG])
    wv = inp("wv", [D, DG])
    bv = inp("bv", [DG])
    ckT = inp("ckT", [DG, PC])      # cache_k[b,:,slice].T
    cv = inp("cv", [PC, DG])        # cache_v[b,:,slice]
    wo = inp("wo", [D, DG])         # wo rows permuted to gather order, cols sliced
    bo = inp("bo", [DG])
    y = nc.dram_tensor("y", [S, DG], F32, kind="ExternalOutput").ap()

    with tile.TileContext(nc) as tc, ExitStack() as ctx:
        res = ctx.enter_context(tc.tile_pool(name="res", bufs=1))
        dram = ctx.enter_context(tc.tile_pool(name="dram", bufs=1, space="DRAM"))

        # tiny whole-kernel residents
        bq_t = res.tile([HD, GH], F16, tag="bq")
        bk_t = res.tile([HD, GH], F16, tag="bk")
        bv_t = res.tile([1, DG], F16, tag="bv")
        bo_t = res.tile([1, DG], F16, tag="bo")
        ones_k = res.tile([HD, 1], F16, tag="ones_k")      # [128,1] ones
        ones_r16 = res.tile([1, HD], F16, tag="ones_r16")  # [1,128] ones
        ones_r32 = res.tile([1, HD], F32, tag="ones_r32")
        nc.sync.dma_start(bq_t[:], bq.rearrange("(m p) -> p m", p=HD))
        nc.sync.dma_start(bk_t[:], bk.rearrange("(m p) -> p m", p=HD))
        nc.sync.dma_start(bv_t[:], bv[None, :])
        nc.sync.dma_start(bo_t[:], bo[None, :])
        nc.vector.memset(ones_k[:], 1.0)
        nc.vector.memset(ones_r16[:], 1.0)
        nc.vector.memset(ones_r32[:], 1.0)

        # collective bounce buffers
        bounce_in = []
        bounce_out = []
        for j in range(GH):
            bounce_in.append(dram.tile([HD, GH, NB], F16, tag=f"bi{j}",
                                       name=f"bi{j}"))
            bounce_out.append(dram.tile([4, HD, GH, NB], F16, tag=f"bg{j}",
                                        name=f"bg{j}"))

        with ExitStack() as c12:
            # phase 1+2 residents
            ph = c12.enter_context(tc.tile_pool(name="ph", bufs=1))
            qT = ph.tile([HD, GH, S], F16, tag="qT")        # [128, 4, 2048]
            kTn = ph.tile([HD, GH, S], F16, tag="kTn")
            ckT_t = ph.tile([HD, GH, PC], F16, tag="ckT")
            cv_t = ph.tile([HD, PC // HD, DG], F16, tag="cv")   # [128, 16, 512]
            vn_t = ph.tile([HD, S // HD, DG], F16, tag="vn")
            nc.sync.dma_start(ckT_t[:], ckT.rearrange("(m p) s -> p m s", p=HD))
            nc.sync.dma_start(cv_t[:], cv.rearrange("(ss p) d -> p ss d", p=HD))

            # ---- phase 1: projections ----
            with tc.tile_pool(name="px", bufs=1) as px, \
                 tc.tile_pool(name="pw", bufs=2) as pw, \
                 tc.tile_pool(name="ps1", bufs=1, space="PSUM") as ps1:
                xres = px.tile([HD, NDC, S], F16, tag="xres")   # 8.4 MB
                xr = xT.rearrange("(kc p) s -> p kc s", p=HD)
                for kq in range(4):
                    nc.sync.dma_start(xres[:, 4 * kq:4 * (kq + 1), :],
                                      xr[:, 4 * kq:4 * (kq + 1), :])
                wvt = px.tile([HD, NDC, DG], F16, tag="wvt")    # 2.1 MB
                nc.sync.dma_start(wvt[:],
                                  wv.rearrange("(kc p) n -> p kc n", p=HD))

                # q pass then k pass: weights stay loaded across the 4 s-blocks
                for wsrc, dst, bias_t, scale in (
                        (wq, qT, bq_t, INV_SQRT_HD), (wk, kTn, bk_t, 1.0)):
                    for m in range(GH):
                        wt = pw.tile([HD, NDC, HD], F16, tag="wqk", name="wt")
                        nc.sync.dma_start(
                            wt[:], wsrc[:, HD * m:HD * (m + 1)].rearrange(
                                "(kc p) n -> p kc n", p=HD))
                        psq = [ps1.tile([HD, NB], F32,
                                        tag=f"pp{4 * (m % 2) + sb}",
                                        name=f"psq{sb}") for sb in range(4)]
                        for kc in range(NDC):
                            for sb in range(4):
                                nc.tensor.matmul(
                                    psq[sb][:], wt[:, kc, :],
                                    xres[:, kc, NB * sb:NB * (sb + 1)],
                                    start=(kc == 0), stop=(kc == NDC - 1))
                        for sb in range(4):
                            nc.scalar.activation(
                                dst[:, m, NB * sb:NB * (sb + 1)], psq[sb][:],
                                AF.Identity, bias=bias_t[:, m:m + 1], scale=scale)

                # v pass (natural layout)
                for ss in range(S // HD):
                    psv = ps1.tile([HD, DG], F32, tag=f"pp{ss % 8}", name="psv")
                    for kc in range(NDC):
                        nc.tensor.matmul(psv[:],
                                         xres[:, kc, HD * ss:HD * (ss + 1)],
                                         wvt[:, kc, :],
                                         start=(kc == 0), stop=False)
                    nc.tensor.matmul(psv[:], ones_r16[:], bv_t[:],
                                     start=False, stop=True)
                    nc.any.tensor_copy(vn_t[:, ss, :], psv[:])

            # ---- phase 2: attention per head + AllGather ----
            with tc.tile_pool(name="p2", bufs=6) as p2, \
                 tc.tile_pool(name="zp", bufs=2) as zp, \
                 tc.tile_pool(name="ap", bufs=2) as apool, \
                 tc.tile_pool(name="ps2", bufs=1, space="PSUM") as ps2:
                for j in range(GH):
                    head_scope = nc.named_scope(f"head{j}")
                    head_scope.__enter__()
                    ahead = apool.tile([HD, GH, NB], F16, tag="ah")
                    for sb in range(4):
                        PA = ps2.tile([HD, NB], F32, tag="PA", name="PA")
                        zacc = zp.tile([HD, NB], F16, tag="z")
                        qTs = qT[:, j, NB * sb:NB * (sb + 1)]
                        for c2 in range(NKC // 2):
                            pss = ps2.tile([HD, 2, NB], F32,
                                           tag=f"psS{c2 % 3}", name="pss")
                            e2 = p2.tile([HD, 2, NB], F16, tag="e")
                            for i in range(2):
                                c = 2 * c2 + i
                                if c < PC // HD:
                                    kt = ckT_t[:, j, HD * c:HD * (c + 1)]
                                else:
                                    cc = c - PC // HD
                                    kt = kTn[:, j, HD * cc:HD * (cc + 1)]
                                nc.tensor.matmul(pss[:, i, :], kt, qTs,
                                                 start=True, stop=True)
                            nc.scalar.activation(e2[:], pss[:], AF.Exp)
                            for i in range(2):
                                c = 2 * c2 + i
                                if c < PC // HD:
                                    vt = cv_t[:, c, HD * j:HD * (j + 1)]
                                else:
                                    vt = vn_t[:, c - PC // HD,
                                              HD * j:HD * (j + 1)]
                                nc.tensor.matmul(PA[:], vt, e2[:, i, :],
                                                 start=(c == 0),
                                                 stop=(c == NKC - 1),
                                                 skip_group_check=True)
                            if c2 == 0:
                                nc.vector.tensor_tensor(zacc[:], e2[:, 0, :],
                                                        e2[:, 1, :], ALU.add)
                            else:
                                nc.vector.tensor_tensor(zacc[:], zacc[:],
                                                        e2[:, 0, :], ALU.add)
                                nc.vector.tensor_tensor(zacc[:], zacc[:],
                                                        e2[:, 1, :], ALU.add)
                        psz = ps2.tile([1, NB], F32, tag="psS0", name="psz")
                        nc.tensor.matmul(psz[:], ones_k[:], zacc[:],
                                         start=True, stop=True)
                        zinv = zp.tile([1, NB], F32, tag="zi")
                        nc.vector.reciprocal_approx_fast(zinv[:], psz[:])
                        psb = ps2.tile([HD, NB], F32, tag="psS1", name="psb")
                        nc.tensor.matmul(psb[:], ones_r32[:], zinv[:],
                                         start=True, stop=True)
                        zb = zp.tile([HD, NB], F32, tag="zb")
                        nc.vector.tensor_copy(zb[:], psb[:])
                        nc.vector.tensor_tensor(ahead[:, sb, :], PA[:], zb[:],
                                                ALU.mult)
                    nc.sync.dma_start(bounce_in[j][:], ahead[:])
                    nc.gpsimd.collective_compute(
                        "AllGather", ALU.bypass, replica_groups=GROUPS,
                        ins=[bounce_in[j].opt()], outs=[bounce_out[j].opt()])
                    head_scope.__exit__(None, None, None)

        # ---- phase 3: output projection (full sequence, 512-col wo slice) ----
        with tc.tile_pool(name="p3", bufs=3) as p3, \
             tc.tile_pool(name="lt3", bufs=1) as ltp, \
             tc.tile_pool(name="wo3", bufs=1) as wop, \
             tc.tile_pool(name="ps3", bufs=1, space="PSUM") as ps3:
            wot = wop.tile([HD, 16, NB], F16, tag="wo")
            nc.sync.dma_start(wot[:], wo.rearrange("(c p) n -> p c n", p=HD))
            # one big load per (j, r): [128, 4, 512] contiguous in the bounce
            lts = []
            for j in range(GH):
                for r in range(4):
                    lt = ltp.tile([HD, GH, NB], F16, tag=f"lt{4 * j + r}",
                                  name=f"lt{4 * j + r}")
                    nc.sync.dma_start(lt[:], bounce_out[j][r])
                    lts.append(lt)
            for m in range(S // HD):
                psO = ps3.tile([HD, NB], F32, tag=f"psO{m % 2}", name="psO")
                for jr in range(16):
                    nc.tensor.matmul(
                        psO[:],
                        lts[jr][:, m // 4, HD * (m % 4):HD * (m % 4 + 1)],
                        wot[:, jr, :],
                        start=(jr == 0), stop=False, skip_group_check=True)
                nc.tensor.matmul(psO[:], ones_r16[:], bo_t[:],
                                 start=False, stop=True, skip_group_check=True)
                ot = p3.tile([HD, NB], F32, tag="ot")
                nc.any.tensor_copy(ot[:], psO[:])
                nc.sync.dma_start(y[HD * m:HD * (m + 1), :], ot[:])

    nc.compile()
    return nc


_BUILT = None


def get_built():
    global _BUILT
    if _BUILT is None:
        _BUILT = build()
    return _BUILT


def make_in_maps(x, cache_k, cache_v, wq, bq, wk, bk, wv, bv, wo, bo):
    x = np.asarray(x)
    cache_k = np.asarray(cache_k)
    cache_v = np.asarray(cache_v)
    wq, bq = np.asarray(wq), np.asarray(bq)
    wk, bk = np.asarray(wk), np.asarray(bk)
    wv, bv = np.asarray(wv), np.asarray(bv)
    wo, bo = np.asarray(wo), np.asarray(bo)

    # permute wo rows to match gather order: lhsT chunk jr=(4j+r) holds head 4r+j
    perm = np.concatenate([
        np.arange(HD * (4 * r + j), HD * (4 * r + j) + HD)
        for j in range(GH) for r in range(4)
    ])
    wo_p = wo[perm, :]

    in_maps = []
    for c in range(8):
        b, g = divmod(c, 4)
        sl = slice(DG * g, DG * (g + 1))
        in_maps.append({
            "xT": np.ascontiguousarray(x[b].T).astype(np.float16),
            "wq": wq[:, sl].astype(np.float16),
            "bq": (bq[sl] * INV_SQRT_HD).astype(np.float16),
            "wk": wk[:, sl].astype(np.float16),
            "bk": bk[sl].astype(np.float16),
            "wv": wv[:, sl].astype(np.float16),
            "bv": bv[sl].astype(np.float16),
            "ckT": np.ascontiguousarray(cache_k[b][:, sl].T).astype(np.float16),
            "cv": cache_v[b][:, sl].astype(np.float16),
            "wo": wo_p[:, sl].astype(np.float16),
            "bo": bo[sl].astype(np.float16),
        })
    return in_maps


def assemble(results):
    out = np.empty((B, S, D), np.float32)
    for c in range(8):
        b, g = divmod(c, 4)
        out[b, :, DG * g:DG * (g + 1)] = results[c]["y"]
    return out


def kernel(**inputs):
    nc = get_built()
    in_maps = make_in_maps(**inputs)
    res = run_bass_kernel_spmd(nc, in_maps, core_ids=list(range(8)))
    return assemble(res.results)



# revision 3
# speedup vs baseline: 1.1650x; 1.1650x over previous
"""Cached self-attention Trainium2 kernel (v4).

Sharding: 8 cores = 2 batches x 4 head-groups. Core c: batch b=c//4, group
g=c%4 owns heads 4g..4g+3 (columns 512g:512g+512 of the q/k/v projections).
Each core projects q/k/v for its heads over the full sequence, runs attention
for its 4 heads, the 4 cores of a batch AllGather the (normalized, transposed)
per-head attention outputs, and each core computes the output projection onto
its 512-column slice of wo (full sequence), so outputs tile the model dim.

v4 restructure (vs v3): per-head software pipeline. Head j+1's q/k projection
matmuls and the out-projection partial sums act as TensorE filler while head
j's attention is paced by the exp (ScalarE) stream, so the PE never idles long
enough to trip the HAM idle-throttle. All inputs are pre-packed on the host so
every DMA is partition-contiguous. The softmax epilogue is moved off ScalarE:
the 1/sqrt(HD) scale is folded into the exp activation, q/k bias adds are DVE
tensor_scalar ops, v/out biases are broadcast tiles added during PSUM
evacuation on DVE. PSUM: 2 rotating 2-bank score groups (exp-paced), PA
double-buffered, 2 projection banks (reused by out-proj chains). Out-proj is
split per-head: heads 0-2 accumulate into an SBUF partial while head 3's
attention still runs; only the final 4-matmul chain waits on the last gather.
"""
import numpy as np
from contextlib import ExitStack

import concourse.bass as bass
import concourse.tile as tile
from concourse import bacc, mybir
from concourse.bass_utils import run_bass_kernel_spmd

B, S, PC, D, H = 2, 2048, 2048, 2048, 16
HD = D // H            # 128 head dim
GH = H // 4            # 4 heads per core
DG = GH * HD           # 512 head-dims per core
NB = 512               # block size
NKC = (PC + S) // HD   # 32 key chunks of 128
NCC = PC // HD         # 16 cached key chunks
NDC = D // HD          # 16 contraction chunks
NSS = S // HD          # 16 new-key chunks
F16 = mybir.dt.float16
F32 = mybir.dt.float32
AF = mybir.ActivationFunctionType
ALU = mybir.AluOpType
INV_SQRT_HD = float(1.0 / np.sqrt(HD))

GROUPS = [[0, 1, 2, 3], [4, 5, 6, 7]]


def build():
    nc = bacc.Bacc("TRN2", target_bir_lowering=False, debug=False, num_devices=8)

    def inp(name, shape, dt=F16):
        return nc.dram_tensor(name, shape, dt, kind="ExternalInput").ap()

    # all host-side pre-packed for contiguous per-partition DMA
    xp = inp("xp", [HD, NDC, S])          # xp[p,kc,s] = x[b][s, kc*128+p]
    wqp = inp("wqp", [HD, GH, NDC, HD])   # [p,j,kc,n] = wq[kc*128+p, 512g+128j+n]
    wkp = inp("wkp", [HD, GH, NDC, HD])
    wvp = inp("wvp", [HD, NDC, DG])       # [p,kc,n] = wv[kc*128+p, sl_n]
    bqp = inp("bqp", [HD, GH], F32)       # [p,j] = bq[sl][128j+p]
    bkp = inp("bkp", [HD, GH], F32)
    bvp = inp("bvp", [1, DG])
    bop = inp("bop", [1, DG])
    ckp = inp("ckp", [HD, GH, PC])        # [p,j,key] = cache_k[b][key, 512g+128j+p]
    cvp = inp("cvp", [HD, NCC, DG])       # [p,ss,d] = cache_v[b][ss*128+p, sl_d]
    wop = inp("wop", [HD, 16, DG])        # rows permuted to gather order, packed
    y = nc.dram_tensor("y", [S, DG], F32, kind="ExternalOutput").ap()

    with tile.TileContext(nc) as tc, ExitStack() as ctx:
        res = ctx.enter_context(tc.tile_pool(name="res", bufs=1))
        dram = ctx.enter_context(tc.tile_pool(name="dram", bufs=1, space="DRAM"))
        ps = ctx.enter_context(tc.tile_pool(name="ps", bufs=1, space="PSUM"))

        # whole-kernel residents
        bq_t = res.tile([HD, GH], F32, tag="bq")
        bk_t = res.tile([HD, GH], F32, tag="bk")
        bv_t = res.tile([1, DG], F16, tag="bv")
        bo_t = res.tile([1, DG], F16, tag="bo")
        ones_k = res.tile([HD, 1], F16, tag="ones_k")      # [128,1] ones
        ones_r16 = res.tile([1, HD], F16, tag="ones_r16")  # [1,128] ones
        ones_r32 = res.tile([1, HD], F32, tag="ones_r32")
        nc.sync.dma_start(bq_t[:], bqp)
        nc.sync.dma_start(bk_t[:], bkp)
        nc.sync.dma_start(bv_t[:], bvp)
        nc.sync.dma_start(bo_t[:], bop)
        nc.vector.memset(ones_k[:], 1.0)
        nc.vector.memset(ones_r16[:], 1.0)
        nc.vector.memset(ones_r32[:], 1.0)

        # broadcast bias tiles (bias varies along the free dim, so build
        # [128, 512] broadcast copies once via ones-matmuls)
        bv_bc = res.tile([HD, DG], F16, tag="bv_bc")
        bo_bc = res.tile([HD, DG], F32, tag="bo_bc")
        psx = ps.tile([HD, DG], F32, tag="pq0", name="ps_bv")
        nc.tensor.matmul(psx[:], ones_r16[:], bv_t[:], start=True, stop=True)
        nc.vector.tensor_copy(bv_bc[:], psx[:])
        psx = ps.tile([HD, DG], F32, tag="pq1", name="ps_bo")
        nc.tensor.matmul(psx[:], ones_r16[:], bo_t[:], start=True, stop=True)
        nc.vector.tensor_copy(bo_bc[:], psx[:])

        # collective bounce buffers
        bounce_in = []
        bounce_out = []
        for j in range(GH):
            bounce_in.append(dram.tile([HD, GH, NB], F16, tag=f"bi{j}",
                                       name=f"bi{j}"))
            bounce_out.append(dram.tile([4, HD, GH, NB], F16, tag=f"bg{j}",
                                        name=f"bg{j}"))

        # long-lived attention pools (heads 0..3)
        hp = ctx.enter_context(tc.tile_pool(name="hp", bufs=2))
        vp = ctx.enter_context(tc.tile_pool(name="vp", bufs=1))
        ep = ctx.enter_context(tc.tile_pool(name="ep", bufs=6))
        zp = ctx.enter_context(tc.tile_pool(name="zp", bufs=2))
        apool = ctx.enter_context(tc.tile_pool(name="ap", bufs=2))

        cv_t = vp.tile([HD, NCC, DG], F16, tag="cv")
        vn_t = vp.tile([HD, NSS, DG], F16, tag="vn")

        pq_cnt = [0]

        def proj_chunk(wt, kc_tiles, dst_ap, bias_ap):
            """One [128, 512] projection chunk: 16 accumulating matmuls +
            DVE evacuation with per-partition bias add."""
            psq = ps.tile([HD, NB], F32, tag=f"pq{pq_cnt[0] % 2}", name="psq")
            pq_cnt[0] += 1
            for kc in range(NDC):
                nc.tensor.matmul(psq[:], wt[:, kc, :], kc_tiles[kc],
                                 start=(kc == 0), stop=(kc == NDC - 1))
            nc.vector.tensor_scalar(dst_ap, psq[:], bias_ap, None, ALU.add)

        def v_chunk(ss, xg):
            psv = ps.tile([HD, DG], F32, tag=f"pq{pq_cnt[0] % 2}", name="psv")
            pq_cnt[0] += 1
            for kc in range(NDC):
                nc.tensor.matmul(psv[:],
                                 xg[kc // 4][:, kc % 4, HD * ss:HD * (ss + 1)],
                                 wvt[:, kc, :],
                                 start=(kc == 0), stop=(kc == NDC - 1))
            # vn = psv + bv (bias varies along free dim -> broadcast add)
            nc.vector.scalar_tensor_tensor(vn_t[:, ss, :], psv[:], 1.0,
                                           bv_bc[:], ALU.mult, ALU.add)

        def attention(j, qT, kT, ckT):
            ahead = apool.tile([HD, GH, NB], F16, tag="ah")
            for sb in range(4):
                PA = ps.tile([HD, NB], F32, tag=f"PA{sb % 2}", name="PA")
                zacc = zp.tile([HD, 2, NB], F16, tag="z")
                qs = qT[:, NB * sb:NB * (sb + 1)]
                for c2 in range(NKC // 2):
                    pss = ps.tile([HD, 2, NB], F32, tag=f"pss{c2 % 2}",
                                  name="pss")
                    e2 = ep.tile([HD, 2, NB], F16, tag="e")
                    for i in range(2):
                        c = 2 * c2 + i
                        if c < NCC:
                            kt = ckT[:, HD * c:HD * (c + 1)]
                        else:
                            kt = kT[:, HD * (c - NCC):HD * (c - NCC + 1)]
                        nc.tensor.matmul(pss[:, i, :], kt, qs,
                                         start=True, stop=True)
                    nc.scalar.activation(e2[:], pss[:], AF.Exp,
                                         scale=INV_SQRT_HD)
                    if c2 == 0:
                        nc.vector.tensor_copy(zacc[:], e2[:])
                    else:
                        nc.vector.tensor_tensor(zacc[:], zacc[:], e2[:],
                                                ALU.add)
                    for i in range(2):
                        c = 2 * c2 + i
                        if c < NCC:
                            vt = cv_t[:, c, HD * j:HD * (j + 1)]
                        else:
                            vt = vn_t[:, c - NCC, HD * j:HD * (j + 1)]
                        nc.tensor.matmul(PA[:], vt, e2[:, i, :],
                                         start=(c == 0), stop=(c == NKC - 1),
                                         skip_group_check=True)
                # softmax denominator: fold halves, cross-partition ones-sum,
                # fast reciprocal, broadcast, normalize during PA evacuation
                zfold = zp.tile([HD, NB], F16, tag="zf")
                nc.vector.tensor_tensor(zfold[:], zacc[:, 0, :], zacc[:, 1, :],
                                        ALU.add)
                psz = ps.tile([1, NB], F32, tag="pss0", name="psz")
                nc.tensor.matmul(psz[:], ones_k[:], zfold[:],
                                 start=True, stop=True)
                zinv = zp.tile([1, NB], F32, tag="zi")
                nc.vector.reciprocal_approx_fast(zinv[:], psz[:])
                psb = ps.tile([HD, NB], F32, tag="pss1", name="psb")
                nc.tensor.matmul(psb[:], ones_r32[:], zinv[:],
                                 start=True, stop=True)
                zb = zp.tile([HD, NB], F32, tag="zb")
                nc.vector.tensor_copy(zb[:], psb[:])
                nc.vector.tensor_tensor(ahead[:, sb, :], PA[:], zb[:],
                                        ALU.mult)
            nc.sync.dma_start(bounce_in[j][:], ahead[:])
            nc.gpsimd.collective_compute(
                "AllGather", ALU.bypass, replica_groups=GROUPS,
                ins=[bounce_in[j].opt()], outs=[bounce_out[j].opt()])

        with ExitStack() as xw:
            xpool = xw.enter_context(tc.tile_pool(name="xp", bufs=1))
            wpool = xw.enter_context(tc.tile_pool(name="wp", bufs=2))
            vwpool = xw.enter_context(tc.tile_pool(name="vw", bufs=1))

            # weights for head 0 first (q0's first matmul needs them), then x
            wq_t = {0: wpool.tile([HD, NDC, HD], F16, tag="wq", name="wq0")}
            wk_t = {0: wpool.tile([HD, NDC, HD], F16, tag="wk", name="wk0")}
            nc.sync.dma_start(wq_t[0][:], wqp[:, 0])
            nc.sync.dma_start(wk_t[0][:], wkp[:, 0])
            xg = []
            for i in range(4):
                t = xpool.tile([HD, 4, S], F16, tag=f"xg{i}", name=f"xg{i}")
                nc.sync.dma_start(t[:], xp[:, 4 * i:4 * (i + 1), :])
                xg.append(t)
            wvt = vwpool.tile([HD, NDC, DG], F16, tag="wvt")
            nc.sync.dma_start(wvt[:], wvp)
            nc.sync.dma_start(cv_t[:], cvp)
            ck_t = {0: hp.tile([HD, PC], F16, tag="ckT", name="ck0")}
            nc.sync.dma_start(ck_t[0][:], ckp[:, 0, :])

            def proj_head(j):
                qT = hp.tile([HD, S], F16, tag="qT", name=f"qT{j}")
                kT = hp.tile([HD, S], F16, tag="kT", name=f"kT{j}")
                for sb in range(4):
                    kcs = [xg[kc // 4][:, kc % 4, NB * sb:NB * (sb + 1)]
                           for kc in range(NDC)]
                    proj_chunk(wq_t[j][:], kcs, qT[:, NB * sb:NB * (sb + 1)],
                               bq_t[:, j:j + 1])
                    proj_chunk(wk_t[j][:], kcs, kT[:, NB * sb:NB * (sb + 1)],
                               bk_t[:, j:j + 1])
                return qT, kT

            def prefetch_head(jn):
                wq_t[jn] = wpool.tile([HD, NDC, HD], F16, tag="wq",
                                      name=f"wq{jn}")
                wk_t[jn] = wpool.tile([HD, NDC, HD], F16, tag="wk",
                                      name=f"wk{jn}")
                nc.sync.dma_start(wq_t[jn][:], wqp[:, jn])
                nc.sync.dma_start(wk_t[jn][:], wkp[:, jn])
                ck_t[jn] = hp.tile([HD, PC], F16, tag="ckT", name=f"ck{jn}")
                nc.sync.dma_start(ck_t[jn][:], ckp[:, jn, :])

            # head 0 projections, then the full v pass
            with nc.named_scope("proj0"):
                qkT = {0: proj_head(0)}
                prefetch_head(1)
            with nc.named_scope("vpass"):
                for ss in range(NSS):
                    v_chunk(ss, xg)

            # heads 0-2: attention + next head's projections as PE filler
            for j in range(3):
                with nc.named_scope(f"head{j}"):
                    attention(j, qkT[j][0], qkT[j][1], ck_t[j][:])
                    qkT[j + 1] = proj_head(j + 1)
                    if j + 2 < GH:
                        prefetch_head(j + 2)

        # phase 3 pools (reuse the x/weight SBUF space released above)
        with tc.tile_pool(name="wo3", bufs=1) as wop_pool, \
             tc.tile_pool(name="lt3", bufs=1) as ltp, \
             tc.tile_pool(name="y12", bufs=1) as y12p, \
             tc.tile_pool(name="yo", bufs=3) as yop:
            wot = wop_pool.tile([HD, 16, NB], F16, tag="wo")
            nc.sync.dma_start(wot[:], wop)
            y12 = y12p.tile([HD, NSS, NB], F16, tag="y12")
            lts = {}
            for j in range(3):
                for r in range(4):
                    lt = ltp.tile([HD, GH, NB], F16, tag=f"lt{4 * j + r}",
                                  name=f"lt{4 * j + r}")
                    nc.sync.dma_start(lt[:], bounce_out[j][r])
                    lts[4 * j + r] = lt

            # head 3: attention while out-proj partials (below) fill the PE
            with nc.named_scope("head3"):
                attention(3, qkT[3][0], qkT[3][1], ck_t[3][:])
            for r in range(4):
                lt = ltp.tile([HD, GH, NB], F16, tag=f"lt{12 + r}",
                              name=f"lt{12 + r}")
                nc.sync.dma_start(lt[:], bounce_out[3][r])
                lts[12 + r] = lt

            # out-proj partials over heads 0-2 (ready long before head 3 ends)
            with nc.named_scope("oproj12"):
                for m in range(NSS):
                    psO = ps.tile([HD, NB], F32, tag=f"pq{m % 2}", name="psO")
                    for jr in range(12):
                        nc.tensor.matmul(
                            psO[:],
                            lts[jr][:, m // 4, HD * (m % 4):HD * (m % 4 + 1)],
                            wot[:, jr, :],
                            start=(jr == 0), stop=(jr == 11),
                            skip_group_check=True)
                    nc.vector.scalar_tensor_tensor(y12[:, m, :], psO[:], 1.0,
                                                   bo_bc[:], ALU.mult, ALU.add)
            # final: head-3 contribution + combine + store
            with nc.named_scope("oproj3"):
                for m in range(NSS):
                    psO = ps.tile([HD, NB], F32, tag=f"pq{m % 2}", name="psO2")
                    for rr in range(4):
                        jr = 12 + rr
                        nc.tensor.matmul(
                            psO[:],
                            lts[jr][:, m // 4, HD * (m % 4):HD * (m % 4 + 1)],
                            wot[:, jr, :],
                            start=(rr == 0), stop=(rr == 3),
                            skip_group_check=True)
                    ot = yop.tile([HD, NB], F32, tag="ot")
                    nc.vector.tensor_tensor(ot[:], psO[:], y12[:, m, :],
                                            ALU.add)
                    nc.sync.dma_start(y[HD * m:HD * (m + 1), :], ot[:])

    nc.compile()
    return nc


_BUILT = None


def get_built():
    global _BUILT
    if _BUILT is None:
        _BUILT = build()
    return _BUILT


def _pack_kc(a):
    """[D, N] -> [128, D//128, N] with [p, kc, n] = a[kc*128+p, n]."""
    d, n = a.shape
    return np.ascontiguousarray(a.reshape(d // HD, HD, n).transpose(1, 0, 2))


def make_in_maps(x, cache_k, cache_v, wq, bq, wk, bk, wv, bv, wo, bo):
    x = np.asarray(x, np.float32)
    cache_k = np.asarray(cache_k, np.float32)
    cache_v = np.asarray(cache_v, np.float32)
    wq, bq = np.asarray(wq, np.float32), np.asarray(bq, np.float32)
    wk, bk = np.asarray(wk, np.float32), np.asarray(bk, np.float32)
    wv, bv = np.asarray(wv, np.float32), np.asarray(bv, np.float32)
    wo, bo = np.asarray(wo, np.float32), np.asarray(bo, np.float32)

    # permute wo rows to match gather order: chunk jr=(4j+r) holds head 4r+j
    perm = np.concatenate([
        np.arange(HD * (4 * r + j), HD * (4 * r + j) + HD)
        for j in range(GH) for r in range(4)
    ])
    wo_p = wo[perm, :]

    # per-batch packed x: [128, 16, S]
    xp_b = [_pack_kc(np.ascontiguousarray(x[b].T)).astype(np.float16)
            for b in range(B)]
    # per-batch packed cache_v rows: [128, 16, D] then slice cols per core
    cv_b = [np.ascontiguousarray(
        cache_v[b].reshape(NCC, HD, D).transpose(1, 0, 2)) for b in range(B)]

    in_maps = []
    for c in range(8):
        b, g = divmod(c, 4)
        sl = slice(DG * g, DG * (g + 1))
        wq_s, wk_s = wq[:, sl], wk[:, sl]
        # [p, j, kc, n]
        wqp = np.ascontiguousarray(
            wq_s.reshape(NDC, HD, GH, HD).transpose(1, 2, 0, 3)
        ).astype(np.float16)
        wkp = np.ascontiguousarray(
            wk_s.reshape(NDC, HD, GH, HD).transpose(1, 2, 0, 3)
        ).astype(np.float16)
        # ckp [p, j, key] = cache_k[b][key, 512g+128j+p]
        ck_s = cache_k[b][:, sl]                      # [PC, 512]
        ckp = np.ascontiguousarray(
            ck_s.reshape(PC, GH, HD).transpose(2, 1, 0)).astype(np.float16)
        in_maps.append({
            "xp": xp_b[b],
            "wqp": wqp,
            "wkp": wkp,
            "wvp": _pack_kc(wv[:, sl]).astype(np.float16),
            "bqp": np.ascontiguousarray(
                bq[sl].reshape(GH, HD).T).astype(np.float32),
            "bkp": np.ascontiguousarray(
                bk[sl].reshape(GH, HD).T).astype(np.float32),
            "bvp": bv[sl][None, :].astype(np.float16),
            "bop": bo[sl][None, :].astype(np.float16),
            "ckp": ckp,
            "cvp": np.ascontiguousarray(cv_b[b][:, :, sl]).astype(np.float16),
            "wop": _pack_kc(wo_p[:, sl]).astype(np.float16),
        })
    return in_maps


def assemble(results):
    out = np.empty((B, S, D), np.float32)
    for c in range(8):
        b, g = divmod(c, 4)
        out[b, :, DG * g:DG * (g + 1)] = results[c]["y"]
    return out


def kernel(**inputs):
    nc = get_built()
    in_maps = make_in_maps(**inputs)
    res = run_bass_kernel_spmd(nc, in_maps, core_ids=list(range(8)))
    return assemble(res.results)
